# revision 27
# baseline (speedup 1.0000x reference)
"""Trainium2 Bass kernel for a pre-LN transformer block.

  x = x + Attn(LN1(x));  out = x + FFN(LN2(x))
  B=128, T=256, E=384, H=6 heads (d=64), FFN hidden 1536, causal, eval mode.

Sharding: data-parallel over batch — 16 batch elements per core x 8 cores.
Weights replicated, no collectives; gather is a host-side concat.

v3: software-pipelined fused chunk loop (8 chunks of 512 tokens = 2 batch
elems).  Engine queues are in-order, so the emission order is staged per
iteration i in per-queue readiness order:

    x load(i+2) | LN1/h1-store/h1T-transposes(i+1) | QKV(i) | attention(i)
    | proj+LN2/h2T(i) | FFN+out(i-1)

which keeps every queue head-of-line free: loads never sit behind stores
that wait on compute, h1T transposes never sit behind h2T ones, and the PE
always has ready matmuls (FFN of i-1, QKV of i) to fill attention
dependency gaps — keeping HAM at full clock.

Other structure:
  - token<->partition maps use the "(p o)" convention (token = 4*p + o) so
    every DRAM transfer (x, h1, h2, out, w2) is per-partition contiguous —
    few large DMA descriptors; proj/FFN2/FFN1 take strided lhsT slices to
    match (stride never costs PE time: LDWEIGHTS is column-count bound).
  - weights DMA in first-use order (wv/wq/wk early, wp/w1/w2 after chunk-0
    prep is queued); x loads on the gpsimd SWDGE ring (fp32->bf16 cast in
    flight), transposes on the sync HWDGE ring, weights on the ACT ring.
  - x resident in SBUF bf16, x2 in fp32 — no second x read, no x2 DRAM
    round trip.
  - per-head merged causal mask multiply ([tri|ones|tri] [128,384] bf16).
  - one shared 8-bank PSUM pool ([128,512] f32) for all matmul groups.
  - LN gains folded into wq/wk/wv/w1 host-side; zero betas/biases elided
    (validated per call); rstd via DVE bit-trick + 2 Newton steps.
"""

import numpy as np
import ml_dtypes

import concourse.bass as bass
import concourse.tile as tile
from concourse import bacc, mybir
from concourse import bass_utils

F32 = mybir.dt.float32
BF16 = mybir.dt.bfloat16
AF = mybir.ActivationFunctionType
OP = mybir.AluOpType

E = 384
H = 6
D = 64
T = 256
NB = 16            # batch elements per core
NT = NB * T        # tokens per core = 4096
NC_CH = NT // 512  # 512-token chunks = 8
NCORES = 8
SCALE = float(E) ** -0.5
EPS = 1e-5
P = 128


def _ln_chunk(nc, small, x_c, h_c, magic_sb):
    """LayerNorm 4 [128, E] tiles (one 512-token chunk) -> bf16 h_c.
    rstd = 1/sqrt(var+eps) computed entirely on DVE (bit-trick seed + 2
    Newton steps): no ACT round trip in the LN chain, and the ACT Exp
    table is never evicted."""
    I32 = mybir.dt.int32
    mv4 = small.tile([P, 4, 2], F32, tag="mv4", name="mv4")
    for t4 in range(4):
        stats = small.tile([P, 6], F32, tag="stats", name="stats")
        nc.vector.bn_stats(out=stats[:], in_=x_c[:, t4, :])
        nc.vector.bn_aggr(out=mv4[:, t4, :], in_=stats[:])
    v4 = small.tile([P, 4], F32, tag="v4", name="v4")
    nc.vector.tensor_scalar_add(v4[:], mv4[:, :, 1], EPS)
    y4 = small.tile([P, 4], F32, tag="y4", name="y4")
    nc.vector.tensor_scalar(
        out=y4.bitcast(I32)[:], in0=v4.bitcast(I32)[:], scalar1=1,
        scalar2=None, op0=OP.arith_shift_right)
    nc.vector.tensor_tensor(
        out=y4.bitcast(I32)[:], in0=magic_sb[:, 0:1].to_broadcast((P, 4)),
        in1=y4.bitcast(I32)[:], op=OP.subtract)
    t4b = small.tile([P, 4], F32, tag="t4b", name="t4b")
    for _ in range(2):  # Newton: y *= 1.5 - 0.5*v*y*y
        nc.vector.tensor_tensor(out=t4b[:], in0=y4[:], in1=y4[:], op=OP.mult)
        nc.vector.tensor_tensor(out=t4b[:], in0=t4b[:], in1=v4[:], op=OP.mult)
        nc.vector.tensor_scalar(
            out=t4b[:], in0=t4b[:], scalar1=-0.5, scalar2=1.5,
            op0=OP.mult, op1=OP.add)
        nc.vector.tensor_tensor(out=y4[:], in0=y4[:], in1=t4b[:], op=OP.mult)
    for t4 in range(4):
        nc.vector.tensor_scalar(
            out=h_c[:, t4, :], in0=x_c[:, t4, :], scalar1=mv4[:, t4, 0:1],
            scalar2=y4[:, t4:t4 + 1], op0=OP.subtract, op1=OP.mult)


def _build_nc():
    nc = bacc.Bacc("TRN2", target_bir_lowering=False, debug=False,
                   num_devices=NCORES)
    x_d = nc.dram_tensor("x", [NT, E], F32, kind="ExternalInput").ap()
    wq_d = nc.dram_tensor("wq", [E, E], BF16, kind="ExternalInput").ap()
    wk_d = nc.dram_tensor("wk", [E, E], BF16, kind="ExternalInput").ap()
    wv_d = nc.dram_tensor("wv", [E, E], BF16, kind="ExternalInput").ap()
    wp_d = nc.dram_tensor("wproj", [E, E], BF16, kind="ExternalInput").ap()
    w1_d = nc.dram_tensor("w1", [E, 4 * E], BF16, kind="ExternalInput").ap()
    w2_d = nc.dram_tensor("w2", [4 * E, E], BF16, kind="ExternalInput").ap()
    b1_d = nc.dram_tensor("b1col", [P, 12], F32, kind="ExternalInput").ap()
    mk_d = nc.dram_tensor("masktri", [P, P], BF16, kind="ExternalInput").ap()
    on_d = nc.dram_tensor("ones64", [P, D], BF16, kind="ExternalInput").ap()
    out_d = nc.dram_tensor("out", [NT, E], F32, kind="ExternalOutput").ap()

    with tile.TileContext(nc) as tc:
        with (
            tc.tile_pool(name="consts", bufs=1) as consts,
            tc.tile_pool(name="dram", bufs=1, space="DRAM") as dram,
            tc.tile_pool(name="small", bufs=6) as small,
            tc.tile_pool(name="xin", bufs=4) as xin,
            tc.tile_pool(name="hcp", bufs=2) as hcp,
            tc.tile_pool(name="hTp", bufs=2) as hTp,
            tc.tile_pool(name="qkvp", bufs=2) as qkvp,
            tc.tile_pool(name="attp", bufs=2) as attp,
            tc.tile_pool(name="x2p", bufs=3) as x2p,
            tc.tile_pool(name="pep", bufs=8) as pep,
            tc.tile_pool(name="rzp", bufs=5) as rzp,
            tc.tile_pool(name="hidp", bufs=2) as hidp,
            tc.tile_pool(name="outp", bufs=2) as outp,
            tc.tile_pool(name="ps", bufs=8, space="PSUM") as ps,
        ):
            magic_sb = consts.tile([P, 1], mybir.dt.int32, tag="magic",
                                   name="magic")
            nc.vector.memset(magic_sb[:], 0x5F3759DF)

            hd1 = [dram.tile([512, E], BF16, name=f"hd1_{c}")
                   for c in range(NC_CH)]
            hd2 = [dram.tile([512, E], BF16, name=f"hd2_{c}")
                   for c in range(NC_CH)]

            first_mm = [None]
            prep_store = [None]
            xts = [None] * NC_CH
            h1Ts = [None] * NC_CH
            qks = [None] * NC_CH
            vs = [None] * NC_CH
            attTs = [None] * NC_CH
            x2s = [None] * NC_CH
            h2Ts = [None] * NC_CH

            def load_x(c):
                # fp32 on the sync HWDGE ring: the gpsimd SWDGE cast-load
                # measured ~3x slower than plain HWDGE for this pattern
                x_c = xin.tile([P, 4, E], F32, tag="x", name="x")
                nc.sync.dma_start(
                    x_c[:], x_d[c * 512:(c + 1) * 512, :]
                    .rearrange("(p o) f -> p o f", o=4))
                xts[c] = x_c

            def prep1(c):
                h_c = hcp.tile([P, 4, E], BF16, tag="h1", name="h1")
                _ln_chunk(nc, small, xts[c], h_c, magic_sb)
                h1T = [hTp.tile([P, 512], BF16, tag=f"h1T{e}", name=f"h1T{e}")
                       for e in range(3)]
                st = nc.gpsimd.dma_start(
                    hd1[c].rearrange("(p o) f -> p o f", o=4), h_c[:])
                if c == 0:
                    prep_store[0] = st
                for e in range(3):
                    nc.sync.dma_start_transpose(
                        h1T[e][:], hd1[c][:, e * P:(e + 1) * P])
                h1Ts[c] = h1T

            def qkv(c):
                h1T = h1Ts[c]
                qk = qkvp.tile([P, 6, 512], BF16, tag="qk", name="qk")
                for hp in range(3):
                    for j, w_sb in enumerate((wq_sb, wk_sb)):
                        psQ = ps.tile([P, 512], F32, tag="ps", name="psq")
                        for k in range(3):
                            mm = nc.tensor.matmul(
                                psQ[:], lhsT=w_sb[:, k, hp * P:(hp + 1) * P],
                                rhs=h1T[k][:], start=(k == 0), stop=(k == 2),
                            )
                            if c == 0 and hp == 0 and j == 0 and k == 0:
                                first_mm[0] = mm
                        nc.scalar.copy(qk[:, 2 * hp + j, :], psQ[:])
                v_sb = qkvp.tile([P, 4, E], BF16, tag="v", name="v")
                for t4 in range(4):
                    psV = ps.tile([P, 512], F32, tag="ps", name="psv")
                    for k in range(3):
                        nc.tensor.matmul(
                            psV[:, 0:E], lhsT=h1T[k][:, t4 * P:(t4 + 1) * P],
                            rhs=wv_sb[:, k, :], start=(k == 0), stop=(k == 2),
                        )
                    nc.vector.tensor_copy(v_sb[:, t4, :], psV[:, 0:E])
                qks[c], vs[c] = qk, v_sb

            def attn(c):
                qk, v_sb = qks[c], vs[c]
                attT = attp.tile([P, 3, 512], BF16, tag="attT", name="attT")
                for b2 in range(2):
                    t0 = b2 * T
                    for hp in range(3):
                        pes = []
                        for r0 in (0, D):
                            lo, hi = r0, r0 + D
                            sc = ps.tile([P, 512], F32, tag="ps", name="sc")
                            nc.tensor.matmul(
                                sc[:, 0:T],
                                lhsT=qk[lo:hi, 2 * hp + 1, t0:t0 + P],
                                rhs=qk[lo:hi, 2 * hp, t0:t0 + T],
                                start=True, stop=True,
                            )
                            nc.tensor.matmul(
                                sc[:, T:3 * P],
                                lhsT=qk[lo:hi, 2 * hp + 1, t0 + P:t0 + T],
                                rhs=qk[lo:hi, 2 * hp, t0 + P:t0 + T],
                                start=True, stop=True,
                            )
                            pe = pep.tile([P, 3 * P], BF16, tag="pe", name="pe")
                            nc.scalar.activation(pe[:], sc[:, 0:3 * P], AF.Exp,
                                                 scale=SCALE)
                            # causal mask: only cols 0:128 (queries 0-127 x
                            # keys 0-127) and 256:384 (queries/keys 128-255)
                            # are triangular; cols 128:256 are all-allowed,
                            # so the mid-block z/a matmuls below never wait
                            # on these two multiplies.
                            nc.vector.tensor_mul(
                                out=pe[:, 0:P], in0=pe[:, 0:P],
                                in1=mk_sb[:])
                            nc.vector.tensor_mul(
                                out=pe[:, T:3 * P], in0=pe[:, T:3 * P],
                                in1=mk_sb[:])
                            pes.append(pe)
                        # z and a in SEPARATE psum banks: matmul start=True
                        # clears the whole bank's has_written bits, so each
                        # bank must keep its start->accumulate sequence
                        # contiguous; splitting lets both mid-block matmuls
                        # (unmasked, exp-only dependency) run first.
                        zA = ps.tile([P, 512], F32, tag="ps", name="za")
                        zZ = ps.tile([P, 512], F32, tag="ps", name="zz")
                        aps = zA[:, 0:T]
                        zps = zZ[:, 0:T]
                        for h2, r0 in enumerate((0, D)):
                            tp = (0, r0)
                            pe = pes[h2]
                            hc = (2 * hp + h2) * D
                            # mid block (keys 0-127 x queries 128-255,
                            # unmasked) first: depends only on exp
                            nc.tensor.matmul(
                                zps[r0:r0 + D, P:T], lhsT=on_sb[:],
                                rhs=pe[:, P:T],
                                start=True, stop=False, tile_position=tp,
                            )
                            nc.tensor.matmul(
                                aps[r0:r0 + D, P:T],
                                lhsT=v_sb[:, 2 * b2, hc:hc + D],
                                rhs=pe[:, P:T],
                                start=True, stop=False, tile_position=tp,
                            )
                            # masked triangular blocks: tri1 accumulates
                            # onto the mid block (no intervening start=True
                            # on the same bank), tri0 is its own group
                            nc.tensor.matmul(
                                zps[r0:r0 + D, P:T], lhsT=on_sb[:],
                                rhs=pe[:, T:3 * P],
                                start=False, stop=True, tile_position=tp,
                            )
                            nc.tensor.matmul(
                                aps[r0:r0 + D, P:T],
                                lhsT=v_sb[:, 2 * b2 + 1, hc:hc + D],
                                rhs=pe[:, T:3 * P],
                                start=False, stop=True, tile_position=tp,
                            )
                            nc.tensor.matmul(
                                zps[r0:r0 + D, 0:P], lhsT=on_sb[:],
                                rhs=pe[:, 0:P],
                                start=True, stop=True, tile_position=tp,
                            )
                            nc.tensor.matmul(
                                aps[r0:r0 + D, 0:P],
                                lhsT=v_sb[:, 2 * b2, hc:hc + D],
                                rhs=pe[:, 0:P],
                                start=True, stop=True, tile_position=tp,
                            )
                        rz = rzp.tile([P, T], F32, tag="rz", name="rz")
                        nc.vector.reciprocal_approx_fast(out=rz[:], in_=zps)
                        nc.vector.tensor_mul(
                            out=attT[:, hp, t0:t0 + T], in0=aps, in1=rz[:],
                        )
                attTs[c] = attT

            def mid(c):
                attT = attTs[c]
                x2_c = x2p.tile([P, 4, E], F32, tag="x2", name="x2")
                for t4 in range(4):
                    psP = ps.tile([P, 512], F32, tag="ps", name="psp")
                    for k in range(3):
                        nc.tensor.matmul(
                            psP[:, 0:E], lhsT=attT[:, k, t4::4],
                            rhs=wp_sb[:, k, :], start=(k == 0), stop=(k == 2),
                        )
                    nc.vector.tensor_add(
                        out=x2_c[:, t4, :], in0=psP[:, 0:E],
                        in1=xts[c][:, t4, :])
                h2_c = hcp.tile([P, 4, E], BF16, tag="h2", name="h2")
                _ln_chunk(nc, small, x2_c, h2_c, magic_sb)
                h2T = [hTp.tile([P, 512], BF16, tag=f"h2T{e}", name=f"h2T{e}")
                       for e in range(3)]
                nc.gpsimd.dma_start(
                    hd2[c].rearrange("(p o) f -> p o f", o=4), h2_c[:])
                for e in range(3):
                    nc.sync.dma_start_transpose(
                        h2T[e][:], hd2[c][:, e * P:(e + 1) * P])
                x2s[c], h2Ts[c] = x2_c, h2T

            def ffn(c):
                h2T, x2_c = h2Ts[c], x2s[c]
                hid_t = hidp.tile([P, 12, 512], BF16, tag="hid", name="hid")
                for m in range(12):
                    psF = ps.tile([P, 512], F32, tag="ps", name="psf")
                    for k in range(3):
                        nc.tensor.matmul(
                            psF[:], lhsT=w1_sb[:, k, m::12],
                            rhs=h2T[k][:], start=(k == 0), stop=(k == 2),
                        )
                    nc.scalar.activation(
                        hid_t[:, m, :], psF[:], AF.Relu,
                        bias=b1_sb[:, m:m + 1], scale=1.0,
                    )
                o_c = outp.tile([P, 4, E], F32, tag="oc", name="oc")
                for t4 in range(4):
                    psO = ps.tile([P, 512], F32, tag="ps", name="pso")
                    for k in range(12):
                        nc.tensor.matmul(
                            psO[:, 0:E], lhsT=hid_t[:, k, t4::4],
                            rhs=w2_sb[:, k, :],
                            start=(k == 0), stop=(k == 11),
                        )
                    nc.vector.tensor_add(
                        out=o_c[:, t4, :], in0=psO[:, 0:E],
                        in1=x2_c[:, t4, :])
                nc.gpsimd.dma_start(
                    out_d[c * 512:(c + 1) * 512, :]
                    .rearrange("(p o) f -> p o f", o=4), o_c[:])

            # ---- prologue ----
            load_x(0)
            wv_sb = consts.tile([P, 3, E], BF16, tag="wv", name="wv")
            nc.scalar.dma_start(wv_sb[:], wv_d.rearrange("(o p) f -> p o f", p=P))
            wq_sb = consts.tile([P, 3, E], BF16, tag="wq", name="wq")
            nc.scalar.dma_start(wq_sb[:], wq_d.rearrange("(o p) f -> p o f", p=P))
            wk_sb = consts.tile([P, 3, E], BF16, tag="wk", name="wk")
            nc.scalar.dma_start(wk_sb[:], wk_d.rearrange("(o p) f -> p o f", p=P))
            mk_sb = consts.tile([P, P], BF16, tag="mk", name="mk")
            nc.scalar.dma_start(mk_sb[:], mk_d)
            on_sb = consts.tile([P, D], BF16, tag="on", name="on")
            nc.scalar.dma_start(on_sb[:], on_d)
            b1_sb = consts.tile([P, 12], F32, tag="b1", name="b1")
            nc.scalar.dma_start(b1_sb[:], b1_d)
            # HAM warmup: dummy matmuls keep the PE busy through the
            # chunk-0 prep chain (x load -> LN1 -> store -> transposes) so
            # the first real matmuls run at full clock instead of paying
            # the 1.2 GHz cold ramp.  Batch 2 is gated on the h1 store so
            # warmth holds until the transposes land.
            wup = ps.tile([P, 512], F32, tag="ps", name="wup")
            for wi in range(55):
                nc.tensor.matmul(
                    wup[:, 0:E], lhsT=wv_sb[:, 0, 0:P], rhs=wv_sb[:, 0, :],
                    start=True, stop=True)
            prep1(0)
            for wi in range(12):
                mm = nc.tensor.matmul(
                    wup[:, 0:E], lhsT=wv_sb[:, 0, 0:P], rhs=wv_sb[:, 0, :],
                    start=True, stop=True)
                if wi == 0:
                    tile.add_dep_helper(
                        mm.ins, prep_store[0].ins, sync=True,
                        reason="warmup batch 2 after chunk-0 h1 store")
            load_x(1)
            wp_sb = consts.tile([P, 3, E], BF16, tag="wp", name="wp")
            w1_sb = consts.tile([P, 3, 4 * E], BF16, tag="w1", name="w1")
            w2_sb = consts.tile([P, 12, E], BF16, tag="w2", name="w2")

            # ---- steady-state pipeline ----
            # wp/w1/w2 DMAs are staggered into iteration 0 so their ~3MB of
            # traffic flows during chunk-0 attention instead of colliding
            # with the chunk-0 h1 store + transposes on the critical path.
            for i in range(NC_CH):
                if i + 2 < NC_CH:
                    load_x(i + 2)
                qkv(i)
                if i == 0:
                    # big weight loads gated on the first chunk-0 matmul so
                    # their ~3MB flows during attention, not on top of the
                    # chunk-0 h1 store / transposes (the scheduler would
                    # otherwise hoist these dependency-free DMAs to t=0)
                    for w_t, w_ap in (
                        (wp_sb, wp_d.rearrange("(o p) f -> p o f", p=P)),
                        (w1_sb, w1_d.rearrange("(o p) f -> p o f", p=P)),
                        (w2_sb, w2_d.rearrange("(p o) f -> p o f", o=12)),
                    ):
                        dma = nc.scalar.dma_start(w_t[:], w_ap)
                        tile.add_dep_helper(
                            dma.ins, first_mm[0].ins, sync=True,
                            reason="weight DMA after chunk-0 prep")
                if i + 1 < NC_CH:
                    prep1(i + 1)
                attn(i)
                mid(i)
                if i >= 1:
                    ffn(i - 1)
            ffn(NC_CH - 1)

    nc.compile()
    return nc


_NC = None
_last_in_maps = None


def _get_nc():
    global _NC
    if _NC is None:
        _NC = _build_nc()
    return _NC


def kernel(x, wq, wk, wv, w_proj, b_proj, w1, b1, w2, b2, g1, beta1, g2, beta2):
    bf16 = ml_dtypes.bfloat16
    x = np.ascontiguousarray(np.asarray(x, np.float32))
    B = x.shape[0]
    g1 = np.asarray(g1, np.float32)
    g2 = np.asarray(g2, np.float32)
    for nm, v in (("beta1", beta1), ("beta2", beta2),
                  ("b_proj", b_proj), ("b2", b2)):
        assert not np.any(np.asarray(v)), (
            f"{nm} != 0 not supported by this build (zero-bias elision)")
    tri = (np.arange(P)[None, :] >= np.arange(P)[:, None])
    consts = {
        # LN gains absorbed into the first-consumer weights (exact)
        "wq": (g1[:, None] * np.asarray(wq, np.float32)).astype(bf16),
        "wk": (g1[:, None] * np.asarray(wk, np.float32)).astype(bf16),
        "wv": (g1[:, None] * np.asarray(wv, np.float32)).astype(bf16),
        "wproj": np.asarray(w_proj, np.float32).astype(bf16),
        "w1": (g2[:, None] * np.asarray(w1, np.float32)).astype(bf16),
        "w2": np.asarray(w2, np.float32).astype(bf16),
        # FFN1 m-tile j holds hidden units {12*p + j} (strided w1 columns),
        # so the bias column layout is just b1.reshape(P, 12)
        "b1col": np.ascontiguousarray(
            np.asarray(b1, np.float32).reshape(P, 12)),
        "masktri": tri.astype(bf16),
        "ones64": np.ones((P, D), dtype=bf16),
    }
    xs = x.reshape(NCORES, NT, E)
    nc = _get_nc()
    in_maps = [dict(consts, x=np.ascontiguousarray(xs[c]))
               for c in range(NCORES)]
    global _last_in_maps
    _last_in_maps = in_maps
    res = bass_utils.run_bass_kernel_spmd(nc, in_maps,
                                          core_ids=list(range(NCORES)))
    out = np.stack([r["out"] for r in res.results], axis=0)
    return out.reshape(B, T, E).astype(np.float32)


if __name__ == "__main__":
    rng = np.random.default_rng(0)
    ins = {
        "x": rng.standard_normal((128, T, E)).astype(np.float32),
        "wq": (rng.standard_normal((E, E)) * E ** -0.5).astype(np.float32),
        "wk": (rng.standard_normal((E, E)) * E ** -0.5).astype(np.float32),
        "wv": (rng.standard_normal((E, E)) * E ** -0.5).astype(np.float32),
        "w_proj": (rng.standard_normal((E, E)) * E ** -0.5).astype(np.float32),
        "b_proj": np.zeros(E, np.float32),
        "w1": (rng.standard_normal((E, 4 * E)) * E ** -0.5).astype(np.float32),
        "b1": np.zeros(4 * E, np.float32),
        "w2": (rng.standard_normal((4 * E, E)) * (4 * E) ** -0.5).astype(np.float32),
        "b2": np.zeros(E, np.float32),
        "g1": np.ones(E, np.float32),
        "beta1": np.zeros(E, np.float32),
        "g2": np.ones(E, np.float32),
        "beta2": np.zeros(E, np.float32),
    }
    out = kernel(**ins)
    print("kernel ran:", out.shape, out.dtype, float(np.abs(out).max()))


# revision 28
# speedup vs baseline: 1.0364x; 1.0364x over previous
"""Trainium2 Bass kernel for a pre-LN transformer block.

  x = x + Attn(LN1(x));  out = x + FFN(LN2(x))
  B=128, T=256, E=384, H=6 heads (d=64), FFN hidden 1536, causal, eval mode.

Sharding: data-parallel over batch — 16 batch elements per core x 8 cores.
Weights replicated, no collectives; gather is a host-side concat.

v3: software-pipelined fused chunk loop (8 chunks of 512 tokens = 2 batch
elems).  Engine queues are in-order, so the emission order is staged per
iteration i in per-queue readiness order:

    x load(i+2) | LN1/h1-store/h1T-transposes(i+1) | QKV(i) | attention(i)
    | proj+LN2/h2T(i) | FFN+out(i-1)

which keeps every queue head-of-line free: loads never sit behind stores
that wait on compute, h1T transposes never sit behind h2T ones, and the PE
always has ready matmuls (FFN of i-1, QKV of i) to fill attention
dependency gaps — keeping HAM at full clock.

Other structure:
  - token<->partition maps use the "(p o)" convention (token = 4*p + o) so
    every DRAM transfer (x, h1, h2, out, w2) is per-partition contiguous —
    few large DMA descriptors; proj/FFN2/FFN1 take strided lhsT slices to
    match (stride never costs PE time: LDWEIGHTS is column-count bound).
  - weights DMA in first-use order (wv/wq/wk early, wp/w1/w2 after chunk-0
    prep is queued); x loads on the gpsimd SWDGE ring (fp32->bf16 cast in
    flight), transposes on the sync HWDGE ring, weights on the ACT ring.
  - x resident in SBUF bf16, x2 in fp32 — no second x read, no x2 DRAM
    round trip.
  - per-head merged causal mask multiply ([tri|ones|tri] [128,384] bf16).
  - one shared 8-bank PSUM pool ([128,512] f32) for all matmul groups.
  - LN gains folded into wq/wk/wv/w1 host-side; zero betas/biases elided
    (validated per call); rstd via DVE bit-trick + 2 Newton steps.
"""

import numpy as np
import ml_dtypes

import concourse.bass as bass
import concourse.tile as tile
from concourse import bacc, mybir
from concourse import bass_utils

F32 = mybir.dt.float32
BF16 = mybir.dt.bfloat16
AF = mybir.ActivationFunctionType
OP = mybir.AluOpType

E = 384
H = 6
D = 64
T = 256
NB = 16            # batch elements per core
NT = NB * T        # tokens per core = 4096
NC_CH = NT // 512  # 512-token chunks = 8
NCORES = 8
SCALE = float(E) ** -0.5
EPS = 1e-5
P = 128


def _ln_chunk(nc, small, x_c, h_c, magic_sb):
    """LayerNorm 4 [128, E] tiles (one 512-token chunk) -> bf16 h_c.
    rstd = 1/sqrt(var+eps) computed entirely on DVE (bit-trick seed + 2
    Newton steps): no ACT round trip in the LN chain, and the ACT Exp
    table is never evicted."""
    I32 = mybir.dt.int32
    mv4 = small.tile([P, 4, 2], F32, tag="mv4", name="mv4")
    for t4 in range(4):
        stats = small.tile([P, 6], F32, tag="stats", name="stats")
        nc.vector.bn_stats(out=stats[:], in_=x_c[:, t4, :])
        nc.vector.bn_aggr(out=mv4[:, t4, :], in_=stats[:])
    v4 = small.tile([P, 4], F32, tag="v4", name="v4")
    nc.vector.tensor_scalar_add(v4[:], mv4[:, :, 1], EPS)
    y4 = small.tile([P, 4], F32, tag="y4", name="y4")
    nc.vector.tensor_scalar(
        out=y4.bitcast(I32)[:], in0=v4.bitcast(I32)[:], scalar1=1,
        scalar2=None, op0=OP.arith_shift_right)
    nc.vector.tensor_tensor(
        out=y4.bitcast(I32)[:], in0=magic_sb[:, 0:1].to_broadcast((P, 4)),
        in1=y4.bitcast(I32)[:], op=OP.subtract)
    t4b = small.tile([P, 4], F32, tag="t4b", name="t4b")
    for _ in range(2):  # Newton: y *= 1.5 - 0.5*v*y*y
        nc.vector.tensor_tensor(out=t4b[:], in0=y4[:], in1=y4[:], op=OP.mult)
        nc.vector.tensor_tensor(out=t4b[:], in0=t4b[:], in1=v4[:], op=OP.mult)
        nc.vector.tensor_scalar(
            out=t4b[:], in0=t4b[:], scalar1=-0.5, scalar2=1.5,
            op0=OP.mult, op1=OP.add)
        nc.vector.tensor_tensor(out=y4[:], in0=y4[:], in1=t4b[:], op=OP.mult)
    for t4 in range(4):
        nc.vector.tensor_scalar(
            out=h_c[:, t4, :], in0=x_c[:, t4, :], scalar1=mv4[:, t4, 0:1],
            scalar2=y4[:, t4:t4 + 1], op0=OP.subtract, op1=OP.mult)


def _build_nc():
    nc = bacc.Bacc("TRN2", target_bir_lowering=False, debug=False,
                   num_devices=NCORES)
    x_d = nc.dram_tensor("x", [NT, E], F32, kind="ExternalInput").ap()
    wq_d = nc.dram_tensor("wq", [E, E], BF16, kind="ExternalInput").ap()
    wk_d = nc.dram_tensor("wk", [E, E], BF16, kind="ExternalInput").ap()
    wv_d = nc.dram_tensor("wv", [E, E], BF16, kind="ExternalInput").ap()
    wp_d = nc.dram_tensor("wproj", [E, E], BF16, kind="ExternalInput").ap()
    w1_d = nc.dram_tensor("w1", [E, 4 * E], BF16, kind="ExternalInput").ap()
    w2_d = nc.dram_tensor("w2", [4 * E, E], BF16, kind="ExternalInput").ap()
    b1_d = nc.dram_tensor("b1col", [P, 12], F32, kind="ExternalInput").ap()
    mk_d = nc.dram_tensor("masktri", [P, P], BF16, kind="ExternalInput").ap()
    on_d = nc.dram_tensor("ones64", [P, D], BF16, kind="ExternalInput").ap()
    out_d = nc.dram_tensor("out", [NT, E], F32, kind="ExternalOutput").ap()

    with tile.TileContext(nc) as tc:
        with (
            tc.tile_pool(name="consts", bufs=1) as consts,
            tc.tile_pool(name="dram", bufs=1, space="DRAM") as dram,
            tc.tile_pool(name="small", bufs=6) as small,
            tc.tile_pool(name="xin", bufs=4) as xin,
            tc.tile_pool(name="hcp", bufs=2) as hcp,
            tc.tile_pool(name="hTp", bufs=2) as hTp,
            tc.tile_pool(name="qkvp", bufs=2) as qkvp,
            tc.tile_pool(name="attp", bufs=2) as attp,
            tc.tile_pool(name="x2p", bufs=3) as x2p,
            tc.tile_pool(name="pep", bufs=6) as pep,
            tc.tile_pool(name="rzp", bufs=4) as rzp,
            tc.tile_pool(name="hidp", bufs=2) as hidp,
            tc.tile_pool(name="outp", bufs=2) as outp,
            tc.tile_pool(name="ps", bufs=8, space="PSUM") as ps,
        ):
            magic_sb = consts.tile([P, 1], mybir.dt.int32, tag="magic",
                                   name="magic")
            nc.vector.memset(magic_sb[:], 0x5F3759DF)

            hd1 = [dram.tile([512, E], BF16, name=f"hd1_{c}")
                   for c in range(NC_CH)]
            hd2 = [dram.tile([512, E], BF16, name=f"hd2_{c}")
                   for c in range(NC_CH)]

            xts = [None] * NC_CH
            h1Ts = [None] * NC_CH
            qks = [None] * NC_CH
            vs = [None] * NC_CH
            attTs = [None] * NC_CH
            x2s = [None] * NC_CH
            h2Ts = [None] * NC_CH

            def load_x(c):
                # fp32 on the sync HWDGE ring: the gpsimd SWDGE cast-load
                # measured ~3x slower than plain HWDGE for this pattern
                x_c = xin.tile([P, 4, E], F32, tag="x", name="x")
                nc.sync.dma_start(
                    x_c[:], x_d[c * 512:(c + 1) * 512, :]
                    .rearrange("(p o) f -> p o f", o=4))
                xts[c] = x_c

            def prep1(c):
                h_c = hcp.tile([P, 4, E], BF16, tag="h1", name="h1")
                _ln_chunk(nc, small, xts[c], h_c, magic_sb)
                h1T = [hTp.tile([P, 512], BF16, tag=f"h1T{e}", name=f"h1T{e}")
                       for e in range(3)]
                nc.gpsimd.dma_start(
                    hd1[c].rearrange("(p o) f -> p o f", o=4), h_c[:])
                for e in range(3):
                    nc.sync.dma_start_transpose(
                        h1T[e][:], hd1[c][:, e * P:(e + 1) * P])
                h1Ts[c] = h1T

            def qkv(c):
                h1T = h1Ts[c]
                qk = qkvp.tile([P, 6, 512], BF16, tag="qk", name="qk")
                for hp in range(3):
                    for j, w_sb in enumerate((wq_sb, wk_sb)):
                        psQ = ps.tile([P, 512], F32, tag="ps", name="psq")
                        for k in range(3):
                            nc.tensor.matmul(
                                psQ[:], lhsT=w_sb[:, k, hp * P:(hp + 1) * P],
                                rhs=h1T[k][:], start=(k == 0), stop=(k == 2),
                            )
                        nc.scalar.copy(qk[:, 2 * hp + j, :], psQ[:])
                v_sb = qkvp.tile([P, 4, E], BF16, tag="v", name="v")
                for t4 in range(4):
                    psV = ps.tile([P, 512], F32, tag="ps", name="psv")
                    for k in range(3):
                        nc.tensor.matmul(
                            psV[:, 0:E], lhsT=h1T[k][:, t4 * P:(t4 + 1) * P],
                            rhs=wv_sb[:, k, :], start=(k == 0), stop=(k == 2),
                        )
                    nc.vector.tensor_copy(v_sb[:, t4, :], psV[:, 0:E])
                qks[c], vs[c] = qk, v_sb

            def attn(c):
                qk, v_sb = qks[c], vs[c]
                attT = attp.tile([P, 3, 512], BF16, tag="attT", name="attT")
                for b2 in range(2):
                    t0 = b2 * T
                    for hp in range(3):
                        pes = []
                        for r0 in (0, D):
                            lo, hi = r0, r0 + D
                            sc = ps.tile([P, 512], F32, tag="ps", name="sc")
                            nc.tensor.matmul(
                                sc[:, 0:T],
                                lhsT=qk[lo:hi, 2 * hp + 1, t0:t0 + P],
                                rhs=qk[lo:hi, 2 * hp, t0:t0 + T],
                                start=True, stop=True,
                            )
                            nc.tensor.matmul(
                                sc[:, T:3 * P],
                                lhsT=qk[lo:hi, 2 * hp + 1, t0 + P:t0 + T],
                                rhs=qk[lo:hi, 2 * hp, t0 + P:t0 + T],
                                start=True, stop=True,
                            )
                            pe = pep.tile([P, 3 * P], BF16, tag="pe", name="pe")
                            nc.scalar.activation(pe[:], sc[:, 0:3 * P], AF.Exp,
                                                 scale=SCALE)
                            # causal mask: only cols 0:128 (queries 0-127 x
                            # keys 0-127) and 256:384 (queries/keys 128-255)
                            # are triangular; cols 128:256 are all-allowed,
                            # so the mid-block z/a matmuls below never wait
                            # on these two multiplies.
                            nc.vector.tensor_mul(
                                out=pe[:, 0:P], in0=pe[:, 0:P],
                                in1=mk_sb[:])
                            nc.vector.tensor_mul(
                                out=pe[:, T:3 * P], in0=pe[:, T:3 * P],
                                in1=mk_sb[:])
                            pes.append(pe)
                        # z and a in SEPARATE psum banks: matmul start=True
                        # clears the whole bank's has_written bits, so each
                        # bank must keep its start->accumulate sequence
                        # contiguous; splitting lets both mid-block matmuls
                        # (unmasked, exp-only dependency) run first.
                        zA = ps.tile([P, 512], F32, tag="ps", name="za")
                        zZ = ps.tile([P, 512], F32, tag="ps", name="zz")
                        aps = zA[:, 0:T]
                        zps = zZ[:, 0:T]
                        for h2, r0 in enumerate((0, D)):
                            tp = (0, r0)
                            pe = pes[h2]
                            hc = (2 * hp + h2) * D
                            # mid block (keys 0-127 x queries 128-255,
                            # unmasked) first: depends only on exp
                            nc.tensor.matmul(
                                zps[r0:r0 + D, P:T], lhsT=on_sb[:],
                                rhs=pe[:, P:T],
                                start=True, stop=False, tile_position=tp,
                            )
                            nc.tensor.matmul(
                                aps[r0:r0 + D, P:T],
                                lhsT=v_sb[:, 2 * b2, hc:hc + D],
                                rhs=pe[:, P:T],
                                start=True, stop=False, tile_position=tp,
                            )
                            # masked triangular blocks: tri1 accumulates
                            # onto the mid block (no intervening start=True
                            # on the same bank), tri0 is its own group
                            nc.tensor.matmul(
                                zps[r0:r0 + D, P:T], lhsT=on_sb[:],
                                rhs=pe[:, T:3 * P],
                                start=False, stop=True, tile_position=tp,
                            )
                            nc.tensor.matmul(
                                aps[r0:r0 + D, P:T],
                                lhsT=v_sb[:, 2 * b2 + 1, hc:hc + D],
                                rhs=pe[:, T:3 * P],
                                start=False, stop=True, tile_position=tp,
                            )
                            nc.tensor.matmul(
                                zps[r0:r0 + D, 0:P], lhsT=on_sb[:],
                                rhs=pe[:, 0:P],
                                start=True, stop=True, tile_position=tp,
                            )
                            nc.tensor.matmul(
                                aps[r0:r0 + D, 0:P],
                                lhsT=v_sb[:, 2 * b2, hc:hc + D],
                                rhs=pe[:, 0:P],
                                start=True, stop=True, tile_position=tp,
                            )
                        rz = rzp.tile([P, T], F32, tag="rz", name="rz")
                        nc.vector.reciprocal_approx_fast(out=rz[:], in_=zps)
                        nc.vector.tensor_mul(
                            out=attT[:, hp, t0:t0 + T], in0=aps, in1=rz[:],
                        )
                attTs[c] = attT

            def mid(c):
                attT = attTs[c]
                x2_c = x2p.tile([P, 4, E], F32, tag="x2", name="x2")
                for t4 in range(4):
                    psP = ps.tile([P, 512], F32, tag="ps", name="psp")
                    for k in range(3):
                        nc.tensor.matmul(
                            psP[:, 0:E], lhsT=attT[:, k, t4::4],
                            rhs=wp_sb[:, k, :], start=(k == 0), stop=(k == 2),
                        )
                    nc.vector.tensor_add(
                        out=x2_c[:, t4, :], in0=psP[:, 0:E],
                        in1=xts[c][:, t4, :])
                h2_c = hcp.tile([P, 4, E], BF16, tag="h2", name="h2")
                _ln_chunk(nc, small, x2_c, h2_c, magic_sb)
                h2T = [hTp.tile([P, 512], BF16, tag=f"h2T{e}", name=f"h2T{e}")
                       for e in range(3)]
                nc.gpsimd.dma_start(
                    hd2[c].rearrange("(p o) f -> p o f", o=4), h2_c[:])
                for e in range(3):
                    nc.sync.dma_start_transpose(
                        h2T[e][:], hd2[c][:, e * P:(e + 1) * P])
                x2s[c], h2Ts[c] = x2_c, h2T

            def ffn(c):
                h2T, x2_c = h2Ts[c], x2s[c]
                hid_t = hidp.tile([P, 12, 512], BF16, tag="hid", name="hid")
                for m in range(12):
                    psF = ps.tile([P, 512], F32, tag="ps", name="psf")
                    for k in range(3):
                        nc.tensor.matmul(
                            psF[:], lhsT=w1_sb[:, k, m::12],
                            rhs=h2T[k][:], start=(k == 0), stop=(k == 2),
                        )
                    nc.scalar.activation(
                        hid_t[:, m, :], psF[:], AF.Relu,
                        bias=b1_sb[:, m:m + 1], scale=1.0,
                    )
                o_c = outp.tile([P, 4, E], F32, tag="oc", name="oc")
                for t4 in range(4):
                    psO = ps.tile([P, 512], F32, tag="ps", name="pso")
                    for k in range(12):
                        nc.tensor.matmul(
                            psO[:, 0:E], lhsT=hid_t[:, k, t4::4],
                            rhs=w2_sb[:, k, :],
                            start=(k == 0), stop=(k == 11),
                        )
                    nc.vector.tensor_add(
                        out=o_c[:, t4, :], in0=psO[:, 0:E],
                        in1=x2_c[:, t4, :])
                nc.gpsimd.dma_start(
                    out_d[c * 512:(c + 1) * 512, :]
                    .rearrange("(p o) f -> p o f", o=4), o_c[:])

            # ---- prologue ----
            load_x(0)
            wv_sb = consts.tile([P, 3, E], BF16, tag="wv", name="wv")
            nc.scalar.dma_start(wv_sb[:], wv_d.rearrange("(o p) f -> p o f", p=P))
            wq_sb = consts.tile([P, 3, E], BF16, tag="wq", name="wq")
            nc.scalar.dma_start(wq_sb[:], wq_d.rearrange("(o p) f -> p o f", p=P))
            wk_sb = consts.tile([P, 3, E], BF16, tag="wk", name="wk")
            nc.scalar.dma_start(wk_sb[:], wk_d.rearrange("(o p) f -> p o f", p=P))
            mk_sb = consts.tile([P, P], BF16, tag="mk", name="mk")
            nc.scalar.dma_start(mk_sb[:], mk_d)
            on_sb = consts.tile([P, D], BF16, tag="on", name="on")
            nc.scalar.dma_start(on_sb[:], on_d)
            b1_sb = consts.tile([P, 12], F32, tag="b1", name="b1")
            nc.scalar.dma_start(b1_sb[:], b1_d)
            prep1(0)
            load_x(1)
            wp_sb = consts.tile([P, 3, E], BF16, tag="wp", name="wp")
            w1_sb = consts.tile([P, 3, 4 * E], BF16, tag="w1", name="w1")
            w2_sb = consts.tile([P, 12, E], BF16, tag="w2", name="w2")

            # ---- steady-state pipeline ----
            # wp/w1/w2 DMAs are staggered into iteration 0 so their ~3MB of
            # traffic flows during chunk-0 attention instead of colliding
            # with the chunk-0 h1 store + transposes on the critical path.
            for i in range(NC_CH):
                if i + 2 < NC_CH:
                    load_x(i + 2)
                qkv(i)
                if i == 0:
                    nc.scalar.dma_start(
                        wp_sb[:], wp_d.rearrange("(o p) f -> p o f", p=P))
                if i + 1 < NC_CH:
                    prep1(i + 1)
                attn(i)
                if i == 0:
                    nc.scalar.dma_start(
                        w1_sb[:], w1_d.rearrange("(o p) f -> p o f", p=P))
                mid(i)
                if i == 0:
                    nc.scalar.dma_start(
                        w2_sb[:], w2_d.rearrange("(p o) f -> p o f", o=12))
                if i >= 1:
                    ffn(i - 1)
            ffn(NC_CH - 1)

    nc.compile()
    return nc


_NC = None
_last_in_maps = None


def _get_nc():
    global _NC
    if _NC is None:
        _NC = _build_nc()
    return _NC


def kernel(x, wq, wk, wv, w_proj, b_proj, w1, b1, w2, b2, g1, beta1, g2, beta2):
    bf16 = ml_dtypes.bfloat16
    x = np.ascontiguousarray(np.asarray(x, np.float32))
    B = x.shape[0]
    g1 = np.asarray(g1, np.float32)
    g2 = np.asarray(g2, np.float32)
    for nm, v in (("beta1", beta1), ("beta2", beta2),
                  ("b_proj", b_proj), ("b2", b2)):
        assert not np.any(np.asarray(v)), (
            f"{nm} != 0 not supported by this build (zero-bias elision)")
    tri = (np.arange(P)[None, :] >= np.arange(P)[:, None])
    consts = {
        # LN gains absorbed into the first-consumer weights (exact)
        "wq": (g1[:, None] * np.asarray(wq, np.float32)).astype(bf16),
        "wk": (g1[:, None] * np.asarray(wk, np.float32)).astype(bf16),
        "wv": (g1[:, None] * np.asarray(wv, np.float32)).astype(bf16),
        "wproj": np.asarray(w_proj, np.float32).astype(bf16),
        "w1": (g2[:, None] * np.asarray(w1, np.float32)).astype(bf16),
        "w2": np.asarray(w2, np.float32).astype(bf16),
        # FFN1 m-tile j holds hidden units {12*p + j} (strided w1 columns),
        # so the bias column layout is just b1.reshape(P, 12)
        "b1col": np.ascontiguousarray(
            np.asarray(b1, np.float32).reshape(P, 12)),
        "masktri": tri.astype(bf16),
        "ones64": np.ones((P, D), dtype=bf16),
    }
    xs = x.reshape(NCORES, NT, E)
    nc = _get_nc()
    in_maps = [dict(consts, x=np.ascontiguousarray(xs[c]))
               for c in range(NCORES)]
    global _last_in_maps
    _last_in_maps = in_maps
    res = bass_utils.run_bass_kernel_spmd(nc, in_maps,
                                          core_ids=list(range(NCORES)))
    out = np.stack([r["out"] for r in res.results], axis=0)
    return out.reshape(B, T, E).astype(np.float32)


if __name__ == "__main__":
    rng = np.random.default_rng(0)
    ins = {
        "x": rng.standard_normal((128, T, E)).astype(np.float32),
        "wq": (rng.standard_normal((E, E)) * E ** -0.5).astype(np.float32),
        "wk": (rng.standard_normal((E, E)) * E ** -0.5).astype(np.float32),
        "wv": (rng.standard_normal((E, E)) * E ** -0.5).astype(np.float32),
        "w_proj": (rng.standard_normal((E, E)) * E ** -0.5).astype(np.float32),
        "b_proj": np.zeros(E, np.float32),
        "w1": (rng.standard_normal((E, 4 * E)) * E ** -0.5).astype(np.float32),
        "b1": np.zeros(4 * E, np.float32),
        "w2": (rng.standard_normal((4 * E, E)) * (4 * E) ** -0.5).astype(np.float32),
        "b2": np.zeros(E, np.float32),
        "g1": np.ones(E, np.float32),
        "beta1": np.zeros(E, np.float32),
        "g2": np.ones(E, np.float32),
        "beta2": np.zeros(E, np.float32),
    }
    out = kernel(**ins)
    print("kernel ran:", out.shape, out.dtype, float(np.abs(out).max()))
